# revision 1
# baseline (speedup 1.0000x reference)
"""Multi-head attention (B=2, S=2048, D=1024, H=16) on 8 trn2 NeuronCores.

Sharding: head-parallel. Core c owns heads {2c, 2c+1} (= feature rows
[128c, 128c+128) of the QKV projections / columns of Wo). Each core:
  - projects full q/k/v (pre-transposed + cast on host) against its
    128-column slice of Wq/Wk/Wv,
  - runs softmax(QK^T * s) @ V for its 4 (batch, head) pairs using a
    transposed-score layout (keys on partitions) so no on-chip transposes
    are needed,
  - computes its additive partial of the output projection
    (attn_heads @ Wo[:, cols].T) in row-parallel fashion.
Host sums the 8 partials and adds the (bo + bv @ Wo.T) constant, which is
where the bv bias lands after the softmax-normalization algebra.

v7: one flat pipeline. All PSUM rides two pools that coexist for the whole
kernel (score/projection/fc ring: 2x[128,1024] = 4 banks; AV accumulators
av0/av1: 4 banks), so there are no pool-transition barriers. The emission
stream interleaves: projection chunks, score+exp steps (the ACT engine's
16.8M exps are the binding resource), AV trailing 2 steps behind its
scores, per-head softmax normalization as soon as that head's AV retires,
and the previous chunk's output projection spread between steps. q/k and
their weights travel fp8e4 (DoubleRow projections, half the input DMA);
the V path stays bf16 for accuracy. Output partials are written bf16.
"""

import sys

for _p in ("/opt/trn_rl_repo",):
    if _p not in sys.path:
        try:
            import concourse  # noqa: F401
            break
        except ImportError:
            sys.path.insert(0, _p)

import numpy as np
import ml_dtypes

import concourse.bass as bass
import concourse.tile as tile
from concourse import mybir
from concourse.bass_utils import run_bass_kernel_spmd

BF16 = mybir.dt.bfloat16
F8 = mybir.dt.float8e4
F32 = mybir.dt.float32
AF = mybir.ActivationFunctionType
DR = mybir.MatmulPerfMode.DoubleRow

B, S, D, H, DH = 2, 2048, 1024, 16, 64
NCORES = 8
T = B * S              # 4096 tokens
HC = 128               # head-columns per core (2 heads x 64)
KO = D // 128          # 8 contraction tiles for projections
SCALE = DH ** -0.5     # 0.125

_NC = None


def _split_multiwaits(nc, maxw=1):
    """Walrus codegen in this container rejects Drain instructions carrying
    more than ~2 semaphore waits ("Too many sync wait commands"). Move the
    excess waits onto preceding NoOps on the same engine."""
    ctr = 0
    for f in nc.m.functions:
        for bb in f.blocks:
            newlist = []
            changed = False
            for inst in bb.instructions:
                si = inst.sync_info
                if (si is not None and si.on_wait and len(si.on_wait) > maxw):
                    waits = list(si.on_wait)
                    for j in range(maxw, len(waits), maxw):
                        nop = mybir.InstNoOp(name=f"splitw-{ctr}", ins=[], outs=[])
                        ctr += 1
                        nop.engine = inst.engine
                        nop.sync_info = mybir.SyncInfo(
                            on_wait=list(waits[j:j + maxw]), on_update=[])
                        newlist.append(nop)
                    inst.sync_info = mybir.SyncInfo(
                        on_wait=waits[:maxw], on_update=list(si.on_update))
                    changed = True
                newlist.append(inst)
            if changed:
                bb.instructions = newlist
    return ctr


def _build(split=True):
    nc = bass.Bass()

    qT = nc.declare_dram_parameter("qT", [D, T], F8, isOutput=False)
    kT = nc.declare_dram_parameter("kT", [D, T], F8, isOutput=False)
    vT = nc.declare_dram_parameter("vT", [D, T], BF16, isOutput=False)
    wq = nc.declare_dram_parameter("wq", [D, HC], F8, isOutput=False)
    wk = nc.declare_dram_parameter("wk", [D, HC], F8, isOutput=False)
    wv = nc.declare_dram_parameter("wv", [D, HC], BF16, isOutput=False)
    bq = nc.declare_dram_parameter("bq", [HC, 1], F32, isOutput=False)
    bk = nc.declare_dram_parameter("bk", [HC, 1], F32, isOutput=False)
    wo = nc.declare_dram_parameter("wo", [HC, D], BF16, isOutput=False)
    ident = nc.declare_dram_parameter("ident", [128, 128], F32, isOutput=False)
    out = nc.declare_dram_parameter("out", [T, D], BF16, isOutput=True)

    qT3 = qT.rearrange("(ko p) n -> p ko n", p=128)
    kT3 = kT.rearrange("(ko p) n -> p ko n", p=128)
    vT3 = vT.rearrange("(ko p) n -> p ko n", p=128)
    wq3 = wq.rearrange("(ko p) m -> p ko m", p=128)
    wk3 = wk.rearrange("(ko p) m -> p ko m", p=128)
    wv3 = wv.rearrange("(ko p) m -> p ko m", p=128)

    NCH = T // 512       # 8 projection chunks of 512 tokens
    # (batch, query-chunk-of-1024) blocks, in token order
    CHUNKS = [(b, qc) for b in range(B) for qc in range(2)]
    # attention steps within a chunk: head 0's k-tiles first (head 1's Q/K
    # rows arrive via a partition-shift DMA, so give them headroom)
    STEPS = [(kt, h) for h in range(2) for kt in range(16)]
    NPOS = len(CHUNKS) * 32          # 128 global score positions
    TRAIL = 2                        # AV trails its scores by this many
    # fc tiles sit late enough that the previous chunk's h1 normalization
    # chain (~8us) has landed in attnT before the PE's in-order queue
    # reaches them, but early enough that their casts retire before the
    # next chunk boundary's normalization burst on the DVE
    FC_AT = {12 + 2 * t: t for t in range(8)}  # in-chunk pos -> fc tile

    with tile.TileContext(nc) as tc:
        with (
            tc.tile_pool(name="consts", bufs=1) as consts,
            tc.tile_pool(name="big", bufs=1) as big,
            tc.tile_pool(name="small", bufs=2) as small,
            tc.tile_pool(name="exps", bufs=6) as exps,
            tc.tile_pool(name="instage", bufs=3) as instage,
            tc.tile_pool(name="scp", bufs=3, space="PSUM") as scp,
            tc.tile_pool(name="avp", bufs=1, space="PSUM") as avp,
            tc.tile_pool(name="dnm", bufs=2, space="DRAM") as dnm,
        ):
            # ---- persistent SBUF state ----
            wq_s = consts.tile([128, KO, 128], F8, tag="wq")
            wk_s = consts.tile([128, KO, 128], F8, tag="wk")
            wv_s = consts.tile([128, KO, 128], BF16, tag="wv")
            wo_s = consts.tile([HC, D], BF16, tag="wo")
            bq_s = consts.tile([HC, 1], F32, tag="bq")
            bk_s = consts.tile([HC, 1], F32, tag="bk")
            id_s = consts.tile([128, 128], F32, tag="id")

            # Per-head Q/K buffers zero-padded to 128 partitions: a K=64
            # matmul runs at half the K=128 streaming rate on this silicon,
            # so scores contract over 128 rows with rows 64-127 always zero.
            QTp = [big.tile([128, T], BF16, tag=f"QTp{h}", name=f"QTp{h}")
                   for h in range(2)]
            KTp = [big.tile([128, T], BF16, tag=f"KTp{h}", name=f"KTp{h}")
                   for h in range(2)]
            attnT = big.tile([HC, T], BF16, tag="attnT")
            # [V | 1] per (batch, local head): k-tokens on partitions,
            # 16 k-tiles x (64 dh + ones) along free.
            V1 = [[big.tile([128, 16 * 65], BF16, tag=f"V1_{b}_{h}",
                            name=f"V1_{b}_{h}")
                   for h in range(2)] for b in range(B)]

            # ---------------- emission helpers ----------------

            qk_stage = {}

            def emit_proj_dmas(c, first=False):
                # q/k land at 1024-token granularity: an fp8 row is only
                # 512B per 512 tokens, too short for full DMA bursts.
                if c % 2 == 0:
                    q2 = instage.tile([128, KO, 1024], F8, tag="q_in")
                    k2 = instage.tile([128, KO, 1024], F8, tag="k_in")
                    cs2 = bass.ds(c * 512, 1024)
                    if first:
                        # land the first contraction tiles ASAP so the first
                        # projection matmul isn't gated on the full fetch
                        nc.sync.dma_start(q2[:, 0:2, :], qT3[:, 0:2, cs2])
                        nc.sync.dma_start(q2[:, 2:KO, :], qT3[:, 2:KO, cs2])
                        nc.sync.dma_start(wk_s[:], wk3[:])
                        nc.sync.dma_start(k2[:, 0:2, :], kT3[:, 0:2, cs2])
                        nc.sync.dma_start(k2[:, 2:KO, :], kT3[:, 2:KO, cs2])
                        nc.sync.dma_start(bq_s[:], bq[:])
                        nc.sync.dma_start(bk_s[:], bk[:])
                    else:
                        nc.sync.dma_start(q2[:], qT3[:, :, cs2])
                        nc.sync.dma_start(k2[:], kT3[:, :, cs2])
                    qk_stage[c // 2] = (q2, k2)
                q2, k2 = qk_stage[c // 2]
                v_in = instage.tile([128, KO, 512], BF16, tag="v_in")
                # issue from GPSIMD: the sync engine's serial DMA-issue
                # stream (~0.6us per descriptor kick) is a phase-A bottleneck
                nc.gpsimd.dma_start(v_in[:], vT3[:, :, bass.ts(c, 512)])
                return q2, k2, (c % 2) * 512, v_in

            def emit_proj_chunk(c, ins):
                """Projections for 512-token chunk c, riding the 3-deep
                shared PSUM ring: one tile for Q|K halves, one for V."""
                cs = bass.ts(c, 512)
                q2, k2, off, v_in = ins

                ps = scp.tile([128, 1024], F32, tag="sp", name=f"psqk{c}")
                for m in range(KO // 2):
                    nc.tensor.matmul(ps[:, 0:512],
                                     wq_s[:, 2 * m: 2 * m + 2, :],
                                     q2[:, 2 * m: 2 * m + 2, off:off + 512],
                                     start=(m == 0), stop=(m == KO // 2 - 1),
                                     perf_mode=DR)
                for m in range(KO // 2):
                    nc.tensor.matmul(ps[:, 512:1024],
                                     wk_s[:, 2 * m: 2 * m + 2, :],
                                     k2[:, 2 * m: 2 * m + 2, off:off + 512],
                                     start=(m == 0), stop=(m == KO // 2 - 1),
                                     perf_mode=DR)
                nc.vector.tensor_scalar_add(QTp[0][0:64, cs], ps[0:64, 0:512],
                                            bq_s[0:64, 0:1])
                stq = small.tile([128, 512], BF16, tag="stq")
                nc.vector.tensor_scalar_add(stq[64:128, :], ps[64:128, 0:512],
                                            bq_s[64:128, 0:1])
                nc.sync.dma_start(QTp[1][0:64, cs], stq[64:128, :])
                nc.vector.tensor_scalar_add(KTp[0][0:64, cs], ps[0:64, 512:1024],
                                            bk_s[0:64, 0:1])
                stk = small.tile([128, 512], BF16, tag="stk")
                nc.vector.tensor_scalar_add(stk[64:128, :], ps[64:128, 512:1024],
                                            bk_s[64:128, 0:1])
                nc.sync.dma_start(KTp[1][0:64, cs], stk[64:128, :])

                # V in natural layout (tokens on partitions); no bias --
                # bv's contribution is folded into the host-side constant.
                # V's PSUM rides the score ring: one alloc per chunk, and
                # only during the projection phase where ACT has slack
                # wv-stationary form (8 weight loads/chunk instead of 32;
                # the token-stationary form was LDWEIGHTS-bound at N=128)
                # into cols 0:512 head-major, then PE identity-transposes
                # flip it token-major into the unused cols 512:1024 of the
                # same ring slot -- no extra PSUM allocation.
                ps_v = scp.tile([128, 1024], F32, tag="sp", name=f"psv{c}")
                for ko in range(KO):
                    nc.tensor.matmul(ps_v[:, 0:512], wv_s[:, ko, :],
                                     v_in[:, ko, :],
                                     start=(ko == 0), stop=(ko == KO - 1))
                vts = small.tile([128, 512], F32, tag="vts")
                nc.scalar.copy(vts[:], ps_v[:, 0:512])
                for sub in range(4):
                    nc.tensor.matmul(ps_v[:, bass.ds(512 + sub * 128, 128)],
                                     vts[:, bass.ts(sub, 128)], id_s[:],
                                     start=True, stop=True,
                                     is_transpose=True)
                for sub in range(4):
                    tok0 = c * 512 + sub * 128
                    b, kt = tok0 // S, (tok0 % S) // 128
                    for h in range(2):
                        nc.vector.tensor_copy(
                            V1[b][h][:, kt * 65: kt * 65 + 64],
                            ps_v[:, 512 + sub * 128 + h * 64:
                                 512 + sub * 128 + (h + 1) * 64])

            e_ring = {}           # (j, kt, h) -> e tile
            av_cur = {}           # j -> [av0, av1]

            def emit_scores_step(j, kt, h):
                b, qc = CHUNKS[j]
                q0 = b * S + qc * 1024
                sp = scp.tile([128, 1024], F32, tag="sp", name=f"sp{j}_{kt}_{h}")
                for half in range(2):
                    nc.tensor.matmul(
                        sp[:, bass.ts(half, 512)],
                        KTp[h][:, bass.ds(b * S + kt * 128, 128)],
                        QTp[h][:, bass.ds(q0 + half * 512, 512)],
                        start=True, stop=True)
                et = exps.tile([128, 1024], BF16, tag="et", name=f"et{j}_{kt}_{h}")
                nc.scalar.activation(et[:], sp[:], AF.Exp, scale=SCALE)
                e_ring[(j, kt, h)] = et

            def emit_av_step(j, kt, h):
                b, qc = CHUNKS[j]
                av = av_cur.setdefault(j, [None, None])
                if kt == 0:
                    # h0-first step order means the two heads' accumulators
                    # are never live at once: one ring-1 tag (2 banks) holds
                    # both, freeing banks for the projection ring
                    av[h] = avp.tile([65, 1024], F32, tag="av",
                                     name=f"av{j}_{h}")
                et = e_ring.pop((j, kt, h))
                for half in range(2):
                    nc.tensor.matmul(
                        av[h][:, bass.ts(half, 512)],
                        V1[b][h][:, kt * 65: kt * 65 + 65],
                        et[:, bass.ts(half, 512)],
                        start=(kt == 0), stop=(kt == 15))

            def emit_norm_head(j, h):
                """Normalize head h of chunk j into attnT, as soon as this
                head's AV accumulation retires (which also frees its PSUM
                accumulator for the next chunk)."""
                b, qc = CHUNKS[j]
                q0 = b * S + qc * 1024
                av = av_cur[j][h]
                avs = small.tile([65, 1024], F32, tag="avs")
                nc.vector.tensor_copy(avs[:], av[:])
                # Reciprocal of the 1024 denominators: a (1, N) op uses
                # a single DVE lane, so bounce through DRAM to respread
                # them over 64 partitions, invert lane-parallel, bounce
                # back, and read back partition-broadcast.
                dscr = dnm.tile([1, 1024], F32, tag="dscr")
                nc.sync.dma_start(dscr[:], avs[64:65, :])
                dsp = small.tile([64, 16], F32, tag="dsp")
                nc.sync.dma_start(
                    dsp[:], dscr.rearrange("o (p j) -> (o p) j", p=64))
                rsp = small.tile([64, 16], F32, tag="rsp")
                nc.vector.reciprocal(rsp[:], dsp[:])
                dscr2 = dnm.tile([1, 1024], F32, tag="dscr2")
                nc.sync.dma_start(
                    dscr2.rearrange("o (p j) -> (o p) j", p=64), rsp[:])
                bcs = small.tile([64, 1024], F32, tag="bcs")
                nc.sync.dma_start(
                    bcs[:], dscr2[0:1, :].to_broadcast((64, 1024)))
                if h == 0:
                    nc.vector.tensor_mul(attnT[0:64, bass.ds(q0, 1024)],
                                         avs[0:64, :], bcs[:])
                else:
                    tmp = small.tile([64, 1024], BF16, tag="tmp")
                    nc.vector.tensor_mul(tmp[:], avs[0:64, :], bcs[:])
                    # partition shift 0-63 -> 64-127 via sbuf DMA
                    nc.sync.dma_start(attnT[64:128, bass.ds(q0, 1024)],
                                      tmp[:])

            def emit_fc_tile(j, tt, last=False):
                """One 128-token tile of the output projection for chunk j,
                riding the "sp" ring (drains via a fast cast)."""
                b, qc = CHUNKS[j]
                t0 = b * S + qc * 1024 + tt * 128
                fp = scp.tile([128, 1024], F32, tag="sp", name=f"fp{j}_{tt}")
                for half in range(2):
                    hs = bass.ts(half, 512)
                    nc.tensor.matmul(fp[:, hs],
                                     attnT[:, bass.ds(t0, 128)],
                                     wo_s[:, hs], start=True, stop=True)
                os_ = small.tile([128, 1024], BF16, tag="os")
                if last and tt % 2 == 0:
                    # tail: both ACT and DVE are idle -- alternate so two
                    # tiles drain in flight
                    nc.scalar.copy(os_[:], fp[:])
                else:
                    nc.vector.tensor_copy(os_[:], fp[:])
                nc.gpsimd.dma_start(out[bass.ds(t0, 128), :], os_[:])

            def emit_attn_pos(p):
                """Emit everything anchored at global score position p:
                the score+exp step, the AV step TRAIL behind, that head's
                normalization when its AV retires, and the previous chunk's
                fc tile at its slot."""
                if p < NPOS:
                    j, s = divmod(p, 32)
                    emit_scores_step(j, *STEPS[s])
                    if s in FC_AT and j >= 1:
                        emit_fc_tile(j - 1, FC_AT[s])
                pa = p - TRAIL
                if 0 <= pa < NPOS:
                    ja, sa = divmod(pa, 32)
                    kt, h = STEPS[sa]
                    emit_av_step(ja, kt, h)
                    if kt == 15:
                        emit_norm_head(ja, h)

            # ---------------- emission schedule ----------------
            nc.sync.dma_start(wq_s[:], wq3[:])
            ins0 = emit_proj_dmas(0, first=True)
            nc.sync.dma_start(wv_s[:], wv3[:])
            ins1 = emit_proj_dmas(1)
            nc.sync.dma_start(wo_s[:], wo[:])
            nc.sync.dma_start(id_s[:], ident[:])
            # one-time SBUF init on the (otherwise idle) GPSIMD engine
            for h in range(2):
                nc.gpsimd.memset(QTp[h][64:128, :], 0.0)
                nc.gpsimd.memset(KTp[h][64:128, :], 0.0)
            for b in range(B):
                for h in range(2):
                    ones_col = V1[b][h].rearrange(
                        "p (t s) -> p t s", s=65)[:, :, 64]
                    nc.gpsimd.memset(ones_col, 1.0)

            emit_proj_chunk(0, ins0)
            emit_proj_chunk(1, ins1)
            # interleave the remaining projections with chunk-0 attention
            steps_per = [6, 6, 6, 5, 5, 4]
            pos = 0
            for c in range(2, NCH):
                emit_proj_chunk(c, emit_proj_dmas(c))
                for _ in range(steps_per[c - 2]):
                    emit_attn_pos(pos)
                    pos += 1
            assert pos == 32
            while pos < NPOS + TRAIL:
                emit_attn_pos(pos)
                pos += 1
            # trailing chunk's output projection
            for tt in range(8):
                emit_fc_tile(len(CHUNKS) - 1, tt, last=True)

    if split:
        _split_multiwaits(nc)
    return nc


def _get_nc():
    global _NC
    if _NC is None:
        _NC = _build()
    return _NC


def _prep_in_maps(q, k, v, Wq, bq, Wk, bk, Wv, bv, Wo, bo):
    bf = ml_dtypes.bfloat16
    f8 = ml_dtypes.float8_e4m3
    qT = np.ascontiguousarray(q.reshape(T, D).T).astype(f8)
    kT = np.ascontiguousarray(k.reshape(T, D).T).astype(f8)
    vT = np.ascontiguousarray(v.reshape(T, D).T).astype(bf)
    in_maps = []
    for c in range(NCORES):
        rows = slice(c * HC, (c + 1) * HC)
        in_maps.append({
            "qT": qT, "kT": kT, "vT": vT,
            "wq": np.ascontiguousarray(Wq[rows, :].T).astype(f8),
            "wk": np.ascontiguousarray(Wk[rows, :].T).astype(f8),
            "wv": np.ascontiguousarray(Wv[rows, :].T).astype(bf),
            "bq": np.ascontiguousarray(bq[rows]).astype(np.float32).reshape(HC, 1),
            "bk": np.ascontiguousarray(bk[rows]).astype(np.float32).reshape(HC, 1),
            "wo": np.ascontiguousarray(Wo[:, rows].T).astype(bf),
            "ident": np.eye(128, dtype=np.float32),
        })
    return in_maps


def _run(inputs, trace=False):
    inputs = {k_: np.asarray(v_) for k_, v_ in inputs.items()}
    nc = _get_nc()
    in_maps = _prep_in_maps(**inputs)
    res = run_bass_kernel_spmd(nc, in_maps, core_ids=list(range(NCORES)),
                               trace=trace)
    acc = np.zeros((T, D), np.float64)
    for c in range(NCORES):
        acc += res.results[c]["out"].astype(np.float64)
    const = (inputs["bo"].astype(np.float64)
             + inputs["bv"].astype(np.float64) @ inputs["Wo"].astype(np.float64).T)
    acc += const[None, :]
    return acc.reshape(B, S, D).astype(np.float32), res


def kernel(**inputs) -> np.ndarray:
    return _run(inputs)[0]

